# revision 8
# baseline (speedup 1.0000x reference)
"""Trainium2 Bass kernel for an LSTM decoder (B=64, T'=127, H=1024).

Strategy: pure data-parallel over the batch dim — each of the 8 cores runs
an independent B=8 LSTM. Per core:
  phase A: embedding gather (transposed, bf16) + input matmul
           pregates^T[4096, 1016] = W_ih' @ X^T  (+ biases), stored bf16
  phase B: 127 serial steps; gates^T = W_hh' @ h^T accumulated over 8
           K-chunks with W tiles stationary; activations on ACT/DVE; h kept
           bf16 in a per-step ring that doubles as the ys output buffer.

Gate-row order is host-permuted to [i, f, o, g] so sigma covers one
contiguous [128, 192] slab and tanh(g) one [128, 64] slab.
"""

import numpy as np
import ml_dtypes

B, T, S, H, V = 64, 128, 128, 1024, 32000
NC = 8
TP = T - 1          # 127 recurrence steps
BL = B // NC        # 8 local batches
NI = TP * BL        # 1016 real gather indices
NIP = 1024          # padded
KC = H // 128       # 8 contraction chunks
MC = 32             # 4096 gate rows / 128
# gate order on device: positions [i, f, o, g] -> original row offsets
OFF = [0, 1024, 3072, 2048]

_CACHED = {}


def _build(t_steps):
    import concourse.bacc as bacc
    import concourse.mybir as mybir
    from concourse import library_config

    dt = mybir.dt
    nc = bacc.Bacc("TRN2", target_bir_lowering=False, debug=False, num_devices=NC)

    # ---------------- DRAM I/O ----------------
    d_emb = nc.dram_tensor("emb", [V, H], dt.bfloat16, kind="ExternalInput").ap()
    d_idx = nc.dram_tensor("idx", [128, NIP // 16], dt.int16, kind="ExternalInput").ap()
    d_wih = nc.dram_tensor("wih", [128, MC * H], dt.bfloat16, kind="ExternalInput").ap()
    d_whh = nc.dram_tensor("whh", [128, MC * H], dt.bfloat16, kind="ExternalInput").ap()
    d_bias = nc.dram_tensor("bias", [128, MC], dt.float32, kind="ExternalInput").ap()
    d_h0 = nc.dram_tensor("h0t", [128, 64], dt.bfloat16, kind="ExternalInput").ap()
    d_c0 = nc.dram_tensor("c0t", [128, 64], dt.float32, kind="ExternalInput").ap()
    d_ys = nc.dram_tensor("ys", [t_steps, 128, 64], dt.float32, kind="ExternalOutput").ap()
    d_cn = nc.dram_tensor("cn", [128, 64], dt.float32, kind="ExternalOutput").ap()

    # ---------------- SBUF ----------------
    s_idx = nc.alloc_sbuf_tensor("s_idx", [128, NIP // 16], dt.int16).ap()
    s_xt = nc.alloc_sbuf_tensor("s_xt", [128, NIP // 128, KC, 128], dt.bfloat16).ap()
    s_whh = nc.alloc_sbuf_tensor("s_whh", [128, MC * H], dt.bfloat16).ap()
    s_wb = [nc.alloc_sbuf_tensor(f"s_wb{i}", [128, H], dt.bfloat16).ap() for i in range(2)]
    s_pg = nc.alloc_sbuf_tensor("s_pg", [128, MC, NIP], dt.bfloat16).ap()
    s_bias = nc.alloc_sbuf_tensor("s_bias", [128, MC], dt.float32).ap()
    s_hb = nc.alloc_sbuf_tensor("s_hb", [128, (t_steps + 1) * 64], dt.bfloat16).ap()
    s_gt = nc.alloc_sbuf_tensor("s_gt", [128, MC, BL], dt.float32).ap()
    s_act = nc.alloc_sbuf_tensor("s_act", [128, 256], dt.float32).ap()
    s_c = nc.alloc_sbuf_tensor("s_c", [128, 64], dt.float32).ap()
    s_t1 = nc.alloc_sbuf_tensor("s_t1", [128, 64], dt.float32).ap()
    s_t2 = nc.alloc_sbuf_tensor("s_t2", [128, 64], dt.float32).ap()
    s_tc = nc.alloc_sbuf_tensor("s_tc", [128, 64], dt.float32).ap()

    # ---------------- PSUM ----------------
    p_in = [nc.alloc_psum_tensor(f"p_in{i}", [128, 512], dt.float32).ap() for i in range(2)]
    p_rec = [nc.alloc_psum_tensor(f"p_rec{i}", [128, MC, BL], dt.float32).ap() for i in range(2)]

    # ---------------- semaphores ----------------
    m_ld = nc.alloc_semaphore("m_ld")      # initial HWDGE loads
    m_idx = nc.alloc_semaphore("m_idx")    # idx load
    m_wihp = [nc.alloc_semaphore(f"m_wih{i}") for i in range(2)]  # W_ih parity
    m_g = nc.alloc_semaphore("m_g")        # gather done
    m_pein = nc.alloc_semaphore("m_pein")  # input matmul groups
    m_actin = nc.alloc_semaphore("m_actin")  # input ACT copies
    m_pe = nc.alloc_semaphore("m_pe")      # recurrence step matmuls
    m_act = nc.alloc_semaphore("m_act")    # recurrence ACT ops
    m_dve = nc.alloc_semaphore("m_dve")    # recurrence DVE ops
    m_fin = nc.alloc_semaphore("m_fin")    # output DMAs
    m_ys = nc.alloc_semaphore("m_ys")      # ys writeback

    NLOADS = 4  # whh, bias, h0, c0

    # ============ SP (sync) engine: loads + final c store ============
    nc.sync.dma_start(s_idx[:, :], d_idx[:, :]).then_inc(m_idx, 16)
    nc.sync.dma_start(s_whh[:, :], d_whh[:, :]).then_inc(m_ld, 16)
    nc.sync.dma_start(s_bias[:, :], d_bias[:, :]).then_inc(m_ld, 16)
    nc.sync.dma_start(s_hb[:, 0:64], d_h0[:, :]).then_inc(m_ld, 16)
    nc.sync.dma_start(s_c[:, :], d_c0[:, :]).then_inc(m_ld, 16)
    # stream W_ih blocks (group g = n*MC + m uses buffer/parity g%2)
    for g in range(2 * MC):
        m = g % MC
        if g >= 2:
            nc.sync.wait_ge(m_pein, g - 1)  # buffer g%2 free after group g-2 retired
        nc.sync.dma_start(s_wb[g % 2][:, :], d_wih[:, m * H:(m + 1) * H]).then_inc(m_wihp[g % 2], 16)
    # final c
    nc.sync.wait_ge(m_dve, 5 * t_steps)
    nc.sync.dma_start(d_cn[:, :], s_c[:, :]).then_inc(m_fin, 16)
    nc.sync.wait_ge(m_fin, 16)

    # ============ GPSIMD: gather + ys writeback ============
    nc.gpsimd.load_library(library_config.mlp)
    nc.gpsimd.wait_ge(m_idx, 16)  # idx present
    for gc in range(NIP // 128):
        nc.gpsimd.dma_gather(s_xt[:, gc, :, :], d_emb[:, :],
                             s_idx[:, gc * 8:(gc + 1) * 8], 128, 128, H,
                             transpose=True).then_inc(m_g, 16)
    nc.gpsimd.wait_ge(m_dve, 5 * t_steps)
    # cast bf16 -> f32 while writing ys
    ys_out = d_ys.rearrange("t p x -> p t x")
    hb_src = s_hb[:, 64:(t_steps + 1) * 64].rearrange("p (t x) -> p t x", x=64)
    nc.gpsimd.dma_start(ys_out, hb_src).then_inc(m_ys, 16)
    nc.gpsimd.wait_ge(m_ys, 16)

    # ============ PE: input matmuls then recurrence ============
    nc.tensor.wait_ge(m_g, 16 * (NIP // 128))
    nc.tensor.wait_ge(m_ld, 16 * NLOADS)
    for g in range(2 * MC):
        n, m = g // MC, g % MC
        ncols = 512
        wbuf = s_wb[g % 2]
        nc.tensor.wait_ge(m_wihp[g % 2], 16 * (g // 2 + 1))
        if g >= 2:
            nc.tensor.wait_ge(m_actin, g - 1)  # psum bank g%2 drained
        for k in range(KC):
            mm = nc.tensor.matmul(
                p_in[g % 2][:, 0:ncols],
                wbuf[:, k * 128:(k + 1) * 128],
                s_xt[:, n * 4:(n + 1) * 4, k, :],
                start=(k == 0), stop=(k == KC - 1))
        mm.then_inc(m_pein, 1)

    # recurrence
    for t in range(t_steps):
        if t > 0:
            nc.tensor.wait_ge(m_dve, 5 * (t - 1) + 5)  # h(t-1) ready
        pr = p_rec[t % 2]
        for m in range(MC):
            for k in range(KC):
                mm = nc.tensor.matmul(
                    pr[:, m, :],
                    s_whh[:, (m * KC + k) * 128:(m * KC + k + 1) * 128],
                    s_hb[:, t * 64 + k * BL: t * 64 + (k + 1) * BL],
                    start=(k == 0), stop=(k == KC - 1))
        mm.then_inc(m_pe, 1)

    # ============ ACT ============
    nc.scalar.wait_ge(m_ld, 16 * NLOADS)
    for g in range(2 * MC):
        n, m = g // MC, g % MC
        ncols = 512
        nc.scalar.wait_ge(m_pein, g + 1)
        nc.scalar.activation(
            s_pg[:, m, n * 512:n * 512 + ncols], p_in[g % 2][:, 0:ncols],
            mybir.ActivationFunctionType.Identity,
            bias=s_bias[:, m:m + 1], scale=1.0).then_inc(m_actin, 1)
    for t in range(t_steps):
        AF = mybir.ActivationFunctionType
        if t > 0:
            nc.scalar.wait_ge(m_act, 3 * t)  # own pipeline drained
        nc.scalar.wait_ge(m_dve, 5 * t + 1)  # s_gt ready
        nc.scalar.activation(s_act[:, 0:192], s_gt[:, 0:24, :], AF.Sigmoid).then_inc(m_act, 1)
        nc.scalar.activation(s_act[:, 192:256], s_gt[:, 24:32, :], AF.Tanh).then_inc(m_act, 1)
        nc.scalar.wait_ge(m_dve, 5 * t + 4)  # c updated
        nc.scalar.activation(s_tc[:, :], s_c[:, :], AF.Tanh).then_inc(m_act, 1)

    # ============ DVE ============
    alu = mybir.AluOpType
    nc.vector.wait_ge(m_ld, 16 * NLOADS)
    for t in range(t_steps):
        if t > 0:
            nc.vector.wait_ge(m_dve, 5 * t)  # own pipeline drained
        nc.vector.wait_ge(m_pe, t + 1)
        if t == 0:
            nc.vector.wait_ge(m_actin, 2 * MC)  # all pregates written
        nc.vector.tensor_tensor(
            s_gt[:, :, :], p_rec[t % 2][:, :, :],
            s_pg[:, :, t * BL:(t + 1) * BL], op=alu.add).then_inc(m_dve, 1)
        nc.vector.wait_ge(m_act, 3 * t + 2)
        nc.vector.tensor_tensor(s_t1[:, :], s_act[:, 0:64], s_act[:, 192:256],
                                op=alu.mult).then_inc(m_dve, 1)
        nc.vector.tensor_tensor(s_t2[:, :], s_act[:, 64:128], s_c[:, :],
                                op=alu.mult).then_inc(m_dve, 1)
        nc.vector.wait_ge(m_dve, 5 * t + 3)  # t1/t2 retired
        nc.vector.tensor_tensor(s_c[:, :], s_t1[:, :], s_t2[:, :],
                                op=alu.add).then_inc(m_dve, 1)
        nc.vector.wait_ge(m_act, 3 * t + 3)
        nc.vector.tensor_tensor(s_hb[:, (t + 1) * 64:(t + 2) * 64],
                                s_act[:, 128:192], s_tc[:, :],
                                op=alu.mult).then_inc(m_dve, 1)

    nc.compile()
    return nc


def _prep_inputs(tgt, h0, c0, emb_table, w_ih, w_hh, b_ih, b_hh, t_steps):
    tgt = np.asarray(tgt)
    h0 = np.asarray(h0, np.float32)
    c0 = np.asarray(c0, np.float32)
    emb = np.array(emb_table, np.float32)
    emb[0] = 0.0
    emb_bf = emb.astype(ml_dtypes.bfloat16)
    w_ih = np.asarray(w_ih, np.float32)
    w_hh = np.asarray(w_hh, np.float32)
    bsum = (np.asarray(b_ih, np.float32) + np.asarray(b_hh, np.float32))

    # permuted gate-row map: prow[m*128+q] = OFF[m//8] + (m%8)*128 + q
    mm = np.arange(MC)
    rows = (np.array(OFF)[mm // 8, None] + (mm % 8)[:, None] * 128
            + np.arange(128)[None, :]).reshape(-1)  # [4096]

    def wt(w):
        # host layout [128 p, m*1024 + k*128 + q] = w[rows[m*128+q], k*128+p]
        wp = w[rows]                                   # [4096, 1024]
        wp = wp.reshape(MC, 128, KC, 128)              # m, q, k, p
        wp = np.transpose(wp, (3, 0, 2, 1))            # p, m, k, q
        return np.ascontiguousarray(wp.reshape(128, MC * H)).astype(ml_dtypes.bfloat16)

    wih_h = wt(w_ih)
    whh_h = wt(w_hh)
    bias_h = np.ascontiguousarray(
        bsum[rows].reshape(MC, 128).T).astype(np.float32)   # [128, MC]

    in_maps = []
    for r in range(NC):
        bsl = slice(r * BL, (r + 1) * BL)
        # gather indices, t-major, padded
        idx = np.zeros(NIP, np.int16)
        idx[:t_steps * BL] = tgt[bsl, :t_steps].T.reshape(-1)
        idxw = np.zeros((16, NIP // 16), np.int16)
        for i in range(NIP):
            idxw[i % 16, i // 16] = idx[i]
        idx128 = np.tile(idxw, (8, 1))
        # h0^T, c0^T in [p, k*8+b] layout
        h0t = np.transpose(h0[0, bsl].reshape(BL, KC, 128), (2, 1, 0)).reshape(128, 64)
        c0t = np.transpose(c0[0, bsl].reshape(BL, KC, 128), (2, 1, 0)).reshape(128, 64)
        in_maps.append({
            "emb": emb_bf,
            "idx": idx128,
            "wih": wih_h,
            "whh": whh_h,
            "bias": bias_h,
            "h0t": np.ascontiguousarray(h0t).astype(ml_dtypes.bfloat16),
            "c0t": np.ascontiguousarray(c0t).astype(np.float32),
        })
    return in_maps


def kernel(tgt, h0, c0, encoder_outputs, src_lengths, emb_table, w_ih, w_hh,
           b_ih, b_hh):
    import concourse.bass_utils as bass_utils

    t_steps = TP
    if "nc" not in _CACHED:
        _CACHED["nc"] = _build(t_steps)
    nc = _CACHED["nc"]
    in_maps = _prep_inputs(tgt, h0, c0, emb_table, w_ih, w_hh, b_ih, b_hh, t_steps)
    res = bass_utils.run_bass_kernel_spmd(nc, in_maps, core_ids=list(range(NC)))
    _CACHED["last_res"] = res

    ys = np.zeros((B, t_steps, H), np.float32)
    cN = np.zeros((B, H), np.float32)
    for r in range(NC):
        bsl = slice(r * BL, (r + 1) * BL)
        yo = res.results[r]["ys"]            # [t, 128 p, 64 (k*8+b)]
        yo = yo.reshape(t_steps, 128, KC, BL)
        ys[bsl] = np.transpose(yo, (3, 0, 2, 1)).reshape(BL, t_steps, H)
        co = res.results[r]["cn"].reshape(128, KC, BL)
        cN[bsl] = np.transpose(co, (2, 1, 0)).reshape(BL, H)
    hN = ys[:, t_steps - 1]
    return ys, hN[None], cN[None]
